# revision 32
# baseline (speedup 1.0000x reference)
"""Trainium2 Bass kernel for nn_Attention_63264868270755.

Full attention layer: QKV projection + rotary + causal attention with XL
memories and a learned null KV + output projection, returning
(out [B,N,INNER], next_xl_memories [2,B,H,1+N,DH]).

Sharding over 8 NeuronCores: data parallel on batch (2 groups of 4 cores),
tensor parallel on heads within each group (4 heads per core). The output
projection produces per-core partials that are summed with an on-chip
ReduceScatter within each 4-core group; each core then owns a distinct
512-row slice of its batch's output.

Device-side layouts (per core, batch b, heads H0..H0+3, head pairs hp=0,1):
  xT   [128, 8, 2048]   x[b] transposed on the PE (dim on partitions)
  qT   [128, 2, 2048]   rotated q, dh on partitions, 2 heads stacked per hp
  krot [128, 2, 2688]   rotated k in permuted kv order [mem(512)|seq(2048)|null|pad]
  vkv  [128, 4, 21, 65] v rows in kv-chunk layout, col 64 = softmax-ones column
Scores are computed transposed (kv on partitions, n free) so exp(sim) feeds
attn@V directly; the denominator comes from the appended ones column of V.

Matmuls run with inputs viewed as float32r (full-rate on the PE; fp32 matmul
is 4x slower) accumulating in fp32 PSUM.
"""

import sys
import types
from contextlib import ExitStack

for _p in ("/opt/trn_rl_repo", "/root/.axon_site"):
    if _p not in sys.path:
        sys.path.insert(0, _p)

import ml_dtypes
import numpy as np

import concourse.bass as bass
import concourse.bass_utils as bass_utils
import concourse.mybir as mybir
import concourse.tile as tile
from concourse import bacc
from concourse.bass_utils import run_bass_kernel_spmd
from concourse.masks import make_identity
from bass_rust import add_dep_helper

FP = mybir.dt.float32
DT_MM = mybir.dt.float32r  # matmul input view; fp32 bits, 1 cycle/row on PE
BF = mybir.dt.bfloat16     # attention operand dtype (QK / exp / AV)

B, N, DIM, H, DH, M = 2, 2048, 1024, 16, 64, 512
INNER = H * DH
KV = M + 1 + N          # 2561 (reference kv order: mem | null | seq)
NCHUNKS = 21
KVP = NCHUNKS * 128     # 2688, padded kv in permuted order: mem | seq | null | pad
NI = 4                  # n tiles of 512
SCALE = DH ** -0.5

_COMPILED = None
_LAST_RESULT = None  # BassKernelResults of the most recent run (for test harness)


def _build_kernel():
    nc = bacc.Bacc()

    x_d = nc.declare_dram_parameter("x", [N, DIM], FP, isOutput=False)
    wq_d = nc.declare_dram_parameter("wq", [DIM, 256], DT_MM, isOutput=False)
    wk_d = nc.declare_dram_parameter("wk", [DIM, 256], DT_MM, isOutput=False)
    wv_d = nc.declare_dram_parameter("wv", [DIM, 256], DT_MM, isOutput=False)
    wout_d = nc.declare_dram_parameter("wout", [256, DIM], DT_MM, isOutput=False)
    kst_d = nc.declare_dram_parameter("k_static", [2, 128, KVP], FP, isOutput=False)
    vst_d = nc.declare_dram_parameter("v_static", [4, 128, NCHUNKS, 65], BF, isOutput=False)
    cosk_d = nc.declare_dram_parameter("cos_k", [128, KVP], FP, isOutput=False)
    sink_d = nc.declare_dram_parameter("sin_k", [128, KVP], FP, isOutput=False)
    cosq_d = nc.declare_dram_parameter("cos_q", [128, N], FP, isOutput=False)
    sinq_d = nc.declare_dram_parameter("sin_q", [128, N], FP, isOutput=False)
    masks_d = nc.declare_dram_parameter("masks", [4, 128, 512], BF, isOutput=False)

    outrs_d = nc.declare_dram_parameter("out_rs", [512, DIM], FP, isOutput=True)
    kout_d = nc.declare_dram_parameter("k_out", [2, 128, N], FP, isOutput=True)
    vout_d = nc.declare_dram_parameter("v_out", [128, 16, 256], FP, isOutput=True)

    with tile.TileContext(nc) as tc, \
         tc.tile_pool(name="persist", bufs=1) as persist, \
         tc.tile_pool(name="dram", bufs=1, space="DRAM") as dram, \
         tc.tile_pool(name="xt", bufs=1) as xtp, \
         tc.tile_pool(name="stream", bufs=2) as stream, \
         tc.tile_pool(name="wpool", bufs=1) as wp, \
         tc.tile_pool(name="tab", bufs=2) as tabp, \
         tc.tile_pool(name="krawp", bufs=1) as krawp, \
         tc.tile_pool(name="exps", bufs=6) as expp, \
         tc.tile_pool(name="nrm", bufs=1) as nrmp, \
         tc.tile_pool(name="ost", bufs=3) as ostp, \
         tc.tile_pool(name="ps", bufs=1, space="PSUM") as ps, \
         tc.tile_pool(name="qkps", bufs=3, space="PSUM") as qkps, \
         tc.tile_pool(name="avps", bufs=4, space="PSUM") as avps:

        qT = persist.tile([128, 2, N], BF)
        krot = persist.tile([128, 2, KVP], BF)
        vkv = persist.tile([128, 4, NCHUNKS, 65], BF)
        aoT = persist.tile([128, 2, N], DT_MM)
        masks_sb = persist.tile([128, 4, 512], BF)
        nc.sync.dma_start(vkv[:], vst_d.rearrange("h p c d -> p h c d"))
        nc.sync.dma_start(masks_sb[:], masks_d.rearrange("d p f -> p d f"))

        part_out = dram.tile([N, DIM], FP)
        rs_out = dram.tile([512, DIM], FP)

        ident = wp.tile([128, 128], FP)
        make_identity(nc, ident)
        wq_sb = wp.tile([128, 8, 256], DT_MM)
        wk_sb = wp.tile([128, 8, 256], DT_MM)
        wv_sb = wp.tile([128, 8, 256], DT_MM)
        wout_sb = wp.tile([128, 2, DIM], DT_MM)
        nc.sync.dma_start(wq_sb[:], wq_d.rearrange("(dc p) c -> p dc c", p=128))
        nc.sync.dma_start(wk_sb[:], wk_d.rearrange("(dc p) c -> p dc c", p=128))
        nc.sync.dma_start(wv_sb[:], wv_d.rearrange("(dc p) c -> p dc c", p=128))
        nc.sync.dma_start(wout_sb[:], wout_d.rearrange("(hp p) n -> p hp n", p=128))

        # PE ordering chain: keeps same-stationary matmuls adjacent and lets us
        # splice projection groups into the attention stream as filler work.
        _pe_chain = [None]

        def chain_pe(mm):
            if _pe_chain[0] is not None:
                add_dep_helper(mm.ins, _pe_chain[0].ins, sync=False,
                               reason="pe order")
            _pe_chain[0] = mm

        # ---- x transpose (PE transpose-mode, 128x128 tiles)
        xT = xtp.tile([128, 8, N], DT_MM)
        _tpools = [(ps, "ps"), (qkps, "qk"), (avps, "av")]
        for nt in range(16):
            xa = stream.tile([128, DIM], FP, tag="xa")
            nc.sync.dma_start(xa[:], x_d[128 * nt:128 * nt + 128, :])
            for dc in range(8):
                pool, tg = _tpools[(nt * 8 + dc) % 3]
                pt = pool.tile([128, 512], FP, tag=tg, name=f"tp{nt}_{dc}")
                nc.tensor.transpose(pt[0:128, 0:128], xa[:, 128 * dc:128 * dc + 128], ident[:])
                nc.vector.tensor_copy(xT[:, dc, 128 * nt:128 * nt + 128], pt[0:128, 0:128])

        def rot_half(dst, src):
            # dst = rotate_half(src) along the dh-partition axis, per head
            for a in (0, 1):
                r = 64 * a
                nc.vector.tensor_scalar_mul(dst[r:r + 32, :], src[r + 32:r + 64, :], -1.0)
                nc.vector.tensor_copy(dst[r + 32:r + 64, :], src[r:r + 32, :])

        # ---- v projection (all 4 heads; also the fp32 next_v output)
        for nt in range(16):
            pv = ps.tile([128, 512], FP, tag="ps", name=f"pv{nt}")
            for dc in range(8):
                mm = nc.tensor.matmul(
                    pv[:, :256],
                    xT[:, dc, 128 * nt:128 * nt + 128].bitcast(DT_MM),
                    wv_sb[:, dc, :].bitcast(DT_MM),
                    start=(dc == 0), stop=(dc == 7),
                )
            for a in range(4):
                nc.vector.tensor_copy(vkv[:, a, 4 + nt, 0:DH], pv[:, 64 * a:64 * a + 64])
            vst = stream.tile([128, 256], FP, tag="vst")
            nc.vector.tensor_copy(vst[:], pv[:, :256])
            nc.sync.dma_start(vout_d[:, nt, :], vst[:])

        def q_group(hp, ni, chained):
            ns = slice(512 * ni, 512 * ni + 512)
            pq = ps.tile([128, 512], FP, tag="ps", name=f"pq{hp}_{ni}")
            for dc in range(8):
                mm = nc.tensor.matmul(
                    pq[:],
                    wq_sb[:, dc, 128 * hp:128 * hp + 128].bitcast(DT_MM),
                    xT[:, dc, ns].bitcast(DT_MM),
                    start=(dc == 0), stop=(dc == 7),
                )
                if chained:
                    chain_pe(mm)
            cos_t = tabp.tile([128, 512], FP, tag="cos")
            sin_t = tabp.tile([128, 512], FP, tag="sin")
            nc.sync.dma_start(cos_t[:], cosq_d[:, ns])
            nc.sync.dma_start(sin_t[:], sinq_d[:, ns])
            rot = stream.tile([128, 512], FP, tag="rot")
            rot_half(rot, pq)
            t1 = stream.tile([128, 512], FP, tag="t1")
            nc.vector.tensor_mul(t1[:], pq[:], cos_t[:])
            nc.vector.tensor_mul(rot[:], rot[:], sin_t[:])
            nc.vector.tensor_add(qT[:, hp, ns], t1[:], rot[:])

        def k_group(hp, ni, kraw, chained):
            ks = slice(512 + 512 * ni, 512 + 512 * ni + 512)
            pk = ps.tile([128, 512], FP, tag="ps", name=f"pk{hp}_{ni}")
            for dc in range(8):
                mm = nc.tensor.matmul(
                    pk[:],
                    wk_sb[:, dc, 128 * hp:128 * hp + 128].bitcast(DT_MM),
                    xT[:, dc, slice(512 * ni, 512 * ni + 512)].bitcast(DT_MM),
                    start=(dc == 0), stop=(dc == 7),
                )
                if chained:
                    chain_pe(mm)
            nc.vector.tensor_copy(kraw[:, ks], pk[:])
            cos_t = tabp.tile([128, 512], FP, tag="cos")
            sin_t = tabp.tile([128, 512], FP, tag="sin")
            nc.sync.dma_start(cos_t[:], cosk_d[:, ks])
            nc.sync.dma_start(sin_t[:], sink_d[:, ks])
            rot = stream.tile([128, 512], FP, tag="rot")
            rot_half(rot, pk)
            t1 = stream.tile([128, 512], FP, tag="t1")
            nc.vector.tensor_mul(t1[:], pk[:], cos_t[:])
            nc.vector.tensor_mul(rot[:], rot[:], sin_t[:])
            nc.vector.tensor_add(krot[:, hp, ks], t1[:], rot[:])

        def k_mem_rotary(hp, kraw):
            # rotary for the mem [0:512] and null+pad [2560:2688] regions, and
            # the raw-k output DMA; DVE/DMA only, no PE work.
            for lo, ln in ((0, 512), (2560, 128)):
                rs = slice(lo, lo + ln)
                cos_t = tabp.tile([128, 512], FP, tag="cos")
                sin_t = tabp.tile([128, 512], FP, tag="sin")
                nc.sync.dma_start(cos_t[:, :ln], cosk_d[:, rs])
                nc.sync.dma_start(sin_t[:, :ln], sink_d[:, rs])
                rot = stream.tile([128, 512], FP, tag="rot")
                rot_half(rot[:, :ln], kraw[:, rs])
                t1 = stream.tile([128, 512], FP, tag="t1")
                nc.vector.tensor_mul(t1[:, :ln], kraw[:, rs], cos_t[:, :ln])
                nc.vector.tensor_mul(rot[:, :ln], rot[:, :ln], sin_t[:, :ln])
                nc.vector.tensor_add(krot[:, hp, rs], t1[:, :ln], rot[:, :ln])
            nc.sync.dma_start(kout_d[hp], kraw[:, 512:512 + N])

        def chunk_nis(c, nis):
            if c == 20:
                return list(nis)
            return [ni for ni in nis if c <= min(4 * ni + 7, 19)]

        def emit_qk_batch(hp, hr, c, nis):
            cs = slice(128 * c, 128 * c + 128)
            exs = {}
            for ni in chunk_nis(c, nis):
                ns = slice(512 * ni, 512 * ni + 512)
                qk = qkps.tile([128, 512], FP, tag="qk", name=f"qk{hp}_{c}_{ni}")
                mm = nc.tensor.matmul(
                    qk[:], krot[hr, hp, cs], qT[hr, hp, ns],
                    start=True, stop=True,
                )
                chain_pe(mm)
                ex = expp.tile([128, 512], BF, tag="ex")
                nc.scalar.activation(
                    ex[:], qk[:], mybir.ActivationFunctionType.Exp, scale=SCALE,
                )
                d = c - 4 * ni
                if 4 <= d <= 7 and c <= 19:
                    nc.vector.tensor_mul(ex[:], ex[:], masks_sb[:, d - 4, :])
                exs[ni] = ex[:]
            return exs

        def emit_av_batch(hp, a, c, av, exs):
            for ni in chunk_nis(c, list(av)):
                mm = nc.tensor.matmul(
                    av[ni][0:65, :],
                    vkv[:, 2 * hp + a, c, :],
                    exs[ni],
                    start=(c == 0), stop=(c == 20),
                    skip_group_check=True,
                )
                chain_pe(mm)

        def attention(hp, a, nis, fillers):
            # one ni-pass (two accumulators) of attention for one head
            hr = slice(64 * a, 64 * a + 64)
            av = {ni: avps.tile([128, 512], FP, tag="av", name=f"av{hp}_{a}_{ni}")
                  for ni in nis}
            last_c = min(4 * max(nis) + 7, 19)
            chunks = list(range(last_c + 1)) + [20]
            pending = []
            for c in chunks:
                exs = emit_qk_batch(hp, hr, c, nis)
                pending.append((c, exs))
                # AV lags two chunks behind QK: the exp of a chunk has two full
                # QK batches of time to complete before its AV needs it
                if len(pending) > 2:
                    cc, ee = pending.pop(0)
                    emit_av_batch(hp, a, cc, av, ee)
                if fillers and c % 2 == 1:
                    fillers.pop(0)()
            for cc, ee in pending:
                emit_av_batch(hp, a, cc, av, ee)
            # paired normalization: one broadcast+reciprocal for both ni
            den = nrmp.tile([1, 1024], FP, tag="den")
            for j, ni in enumerate(nis):
                nc.vector.tensor_copy(den[0:1, 512 * j:512 * j + 512], av[ni][64:65, :])
            bc = nrmp.tile([64, 1024], FP, tag="bc")
            nc.gpsimd.partition_broadcast(bc[:], den[0:1, :])
            rec = nrmp.tile([64, 1024], FP, tag="rec")
            nc.vector.reciprocal(rec[:], bc[:])
            for j, ni in enumerate(nis):
                ns = slice(512 * ni, 512 * ni + 512)
                nc.vector.tensor_mul(
                    aoT[64 * a:64 * a + 64, hp, ns], av[ni][0:64, :],
                    rec[:, 512 * j:512 * j + 512]
                )

        # head pair 0 projections (unchained: PE is otherwise idle here)
        kraw0 = krawp.tile([128, KVP], FP, tag="kraw", name="kraw0")
        nc.sync.dma_start(kraw0[:], kst_d[0])
        for ni in range(NI):
            q_group(0, ni, chained=False)
        for ni in range(NI):
            k_group(0, ni, kraw0, chained=False)
        k_mem_rotary(0, kraw0)

        # head pair 1 projections are spliced into attention(0, *) as fillers
        kraw1 = krawp.tile([128, KVP], FP, tag="kraw", name="kraw1")
        nc.sync.dma_start(kraw1[:], kst_d[1])
        fillers = [lambda ni=ni: q_group(1, ni, chained=True) for ni in range(NI)]
        fillers += [lambda ni=ni: k_group(1, ni, kraw1, chained=True) for ni in range(NI)]

        def out_proj_half(half):
            # output projection for rows [1024*half, 1024*half + 1024) plus the
            # ReduceScatter of that half; the first half's RS overlaps pass B.
            for nt in range(8 * half, 8 * half + 8):
                for ncol in range(2):
                    po = qkps.tile([128, 512], FP, tag="qk", name=f"po{nt}_{ncol}")
                    for hp in range(2):
                        nc.tensor.matmul(
                            po[:],
                            aoT[:, hp, 128 * nt:128 * nt + 128].bitcast(DT_MM),
                            wout_sb[:, hp, 512 * ncol:512 * ncol + 512].bitcast(DT_MM),
                            start=(hp == 0), stop=(hp == 1),
                        )
                    ot = ostp.tile([128, 512], FP, tag="ot")
                    nc.vector.tensor_copy(ot[:], po[:])
                    nc.sync.dma_start(
                        part_out[128 * nt:128 * nt + 128, 512 * ncol:512 * ncol + 512],
                        ot[:],
                    )
            nc.gpsimd.collective_compute(
                "ReduceScatter",
                mybir.AluOpType.add,
                replica_groups=[[0, 1, 2, 3], [4, 5, 6, 7]],
                ins=[part_out[1024 * half:1024 * half + 1024, :]],
                outs=[rs_out[256 * half:256 * half + 256, :]],
            )
            nc.sync.dma_start(outrs_d[256 * half:256 * half + 256, :],
                              rs_out[256 * half:256 * half + 256, :])

        # pass A: ni 0,1 for all heads (fillers: head-pair-1 projections)
        attention(0, 0, (0, 1), fillers)
        attention(0, 1, (0, 1), fillers)
        k_mem_rotary(1, kraw1)
        attention(1, 0, (0, 1), fillers)
        attention(1, 1, (0, 1), fillers)
        out_proj_half(0)
        # pass B: ni 2,3 (RS of half 0 overlaps this pass)
        attention(0, 0, (2, 3), fillers)
        attention(0, 1, (2, 3), fillers)
        attention(1, 0, (2, 3), fillers)
        attention(1, 1, (2, 3), fillers)
        out_proj_half(1)

    nc.compile()
    return nc


def _host_inputs(x, rotary_q, rotary_k, xl_memories, Wq, Wkv, Wout, null_kv):
    """Build the 8 per-core input dicts."""
    x = np.ascontiguousarray(x, np.float32)
    cos_q_T = np.cos(rotary_q).T.astype(np.float32)   # [64, 2048]
    sin_q_T = np.sin(rotary_q).T.astype(np.float32)
    # permuted kv order: mem(512) | seq(2048) | null(1) | pad(127)
    angles = np.concatenate(
        [rotary_k[0:M], rotary_k[M + 1:KV], rotary_k[M:M + 1],
         np.zeros((KVP - KV, DH), np.float32)], axis=0)
    cos_k_T = np.cos(angles).T.astype(np.float32)     # [64, 2688]
    sin_k_T = np.sin(angles).T.astype(np.float32)
    cos_k_T[:, KV:] = 0.0
    sin_k_T[:, KV:] = 0.0
    cos_q2 = np.concatenate([cos_q_T, cos_q_T], axis=0)  # [128, 2048]
    sin_q2 = np.concatenate([sin_q_T, sin_q_T], axis=0)
    cos_k2 = np.concatenate([cos_k_T, cos_k_T], axis=0)  # [128, 2688]
    sin_k2 = np.concatenate([sin_k_T, sin_k_T], axis=0)

    p = np.arange(128)[:, None]
    f = np.arange(512)[None, :]
    masks = np.stack(
        [(512 - 128 * d - p + f >= 0).astype(np.float32) for d in range(4, 8)])

    ins = []
    for c in range(8):
        b, hg = c // 4, c % 4
        H0 = 4 * hg
        k_static = np.zeros((2, 128, KVP), np.float32)
        v_static = np.zeros((4, 128, NCHUNKS, 65), np.float32)
        for hp in range(2):
            for a in range(2):
                h = H0 + 2 * hp + a
                r = slice(64 * a, 64 * a + 64)
                k_static[hp, r, 0:M] = xl_memories[0][b, h].T
                k_static[hp, r, M + N] = null_kv[0][h]
        for a in range(4):
            h = H0 + a
            v_static[a, :, :, 64] = 1.0
            v_static[a, :, 20, :] = 0.0
            v_static[a, :, 0:4, 0:DH] = (
                xl_memories[1][b, h].reshape(4, 128, DH).transpose(1, 0, 2))
            v_static[a, 0, 20, 0:DH] = null_kv[1][h]
            v_static[a, 0, 20, 64] = 1.0
        cs = slice(64 * H0, 64 * H0 + 256)
        ins.append({
            "x": x[b],
            "wq": np.ascontiguousarray(Wq[:, cs], np.float32),
            "wk": np.ascontiguousarray(Wkv[:, 0:INNER][:, cs], np.float32),
            "wv": np.ascontiguousarray(Wkv[:, INNER:][:, cs], np.float32),
            "wout": np.ascontiguousarray(Wout[cs, :], np.float32),
            "k_static": k_static,
            "v_static": v_static.astype(ml_dtypes.bfloat16),
            "cos_k": cos_k2, "sin_k": sin_k2,
            "cos_q": cos_q2, "sin_q": sin_q2,
            "masks": masks.astype(ml_dtypes.bfloat16),
        })
    return ins


def kernel(x, rotary_q, rotary_k, xl_memories, Wq, Wkv, Wout, null_kv):
    global _COMPILED
    x = np.asarray(x, np.float32)
    rotary_q = np.asarray(rotary_q, np.float32)
    rotary_k = np.asarray(rotary_k, np.float32)
    xl_memories = np.asarray(xl_memories, np.float32)
    Wq = np.asarray(Wq, np.float32)
    Wkv = np.asarray(Wkv, np.float32)
    Wout = np.asarray(Wout, np.float32)
    null_kv = np.asarray(null_kv, np.float32)

    if _COMPILED is None:
        _COMPILED = _build_kernel()
    nc = _COMPILED

    ins = _host_inputs(x, rotary_q, rotary_k, xl_memories, Wq, Wkv, Wout, null_kv)
    global _LAST_RESULT
    _LAST_RESULT = run_bass_kernel_spmd(nc, ins, list(range(8)))
    res = _LAST_RESULT.results

    out = np.empty((B, N, INNER), np.float32)
    next_k = np.empty((B, H, 1 + N, DH), np.float32)
    next_v = np.empty((B, H, 1 + N, DH), np.float32)
    next_k[:, :, 0, :] = null_kv[0][None]
    next_v[:, :, 0, :] = null_kv[1][None]
    for c in range(8):
        b, hg = c // 4, c % 4
        H0 = 4 * hg
        orow = res[c]["out_rs"]
        out[b, 256 * hg:256 * hg + 256, :] = orow[0:256]
        out[b, 1024 + 256 * hg:1024 + 256 * hg + 256, :] = orow[256:512]
        ko = res[c]["k_out"].reshape(2, 2, DH, N)     # [hp, a, d, n]
        for hp in range(2):
            for a in range(2):
                next_k[b, H0 + 2 * hp + a, 1:, :] = ko[hp, a].T
        vo = res[c]["v_out"]                          # [128, 16, 4*64]
        for a in range(4):
            next_v[b, H0 + a, 1:, :] = (
                vo[:, :, 64 * a:64 * a + 64].transpose(1, 0, 2).reshape(N, DH))
    return out, np.stack([next_k, next_v])


# revision 33
# speedup vs baseline: 1.0892x; 1.0892x over previous
"""Trainium2 Bass kernel for nn_Attention_63264868270755.

Full attention layer: QKV projection + rotary + causal attention with XL
memories and a learned null KV + output projection, returning
(out [B,N,INNER], next_xl_memories [2,B,H,1+N,DH]).

Sharding over 8 NeuronCores: data parallel on batch (2 groups of 4 cores),
tensor parallel on heads within each group (4 heads per core). The output
projection produces per-core partials that are summed with an on-chip
ReduceScatter within each 4-core group; each core then owns a distinct
512-row slice of its batch's output.

Device-side layouts (per core, batch b, heads H0..H0+3, head pairs hp=0,1):
  xT   [128, 8, 2048]   x[b] transposed on the PE (dim on partitions)
  qT   [128, 2, 2048]   rotated q, dh on partitions, 2 heads stacked per hp
  krot [128, 2, 2688]   rotated k in permuted kv order [mem(512)|seq(2048)|null|pad]
  vkv  [128, 4, 21, 65] v rows in kv-chunk layout, col 64 = softmax-ones column
Scores are computed transposed (kv on partitions, n free) so exp(sim) feeds
attn@V directly; the denominator comes from the appended ones column of V.

Matmuls run with inputs viewed as float32r (full-rate on the PE; fp32 matmul
is 4x slower) accumulating in fp32 PSUM.
"""

import sys
import types
from contextlib import ExitStack

for _p in ("/opt/trn_rl_repo", "/root/.axon_site"):
    if _p not in sys.path:
        sys.path.insert(0, _p)

import ml_dtypes
import numpy as np

import concourse.bass as bass
import concourse.bass_utils as bass_utils
import concourse.mybir as mybir
import concourse.tile as tile
from concourse import bacc
from concourse.bass_utils import run_bass_kernel_spmd
from concourse.masks import make_identity
from bass_rust import add_dep_helper

FP = mybir.dt.float32
DT_MM = mybir.dt.float32r  # matmul input view; fp32 bits, 1 cycle/row on PE
BF = mybir.dt.bfloat16     # attention operand dtype (QK / exp / AV)

B, N, DIM, H, DH, M = 2, 2048, 1024, 16, 64, 512
INNER = H * DH
KV = M + 1 + N          # 2561 (reference kv order: mem | null | seq)
NCHUNKS = 21
KVP = NCHUNKS * 128     # 2688, padded kv in permuted order: mem | seq | null | pad
NI = 4                  # n tiles of 512
SCALE = DH ** -0.5

_COMPILED = None
_LAST_RESULT = None  # BassKernelResults of the most recent run (for test harness)


def _build_kernel():
    nc = bacc.Bacc()

    x_d = nc.declare_dram_parameter("x", [N, DIM], FP, isOutput=False)
    wq_d = nc.declare_dram_parameter("wq", [DIM, 256], DT_MM, isOutput=False)
    wk_d = nc.declare_dram_parameter("wk", [DIM, 256], DT_MM, isOutput=False)
    wv_d = nc.declare_dram_parameter("wv", [DIM, 256], DT_MM, isOutput=False)
    wout_d = nc.declare_dram_parameter("wout", [256, DIM], DT_MM, isOutput=False)
    kst_d = nc.declare_dram_parameter("k_static", [2, 128, KVP], FP, isOutput=False)
    vst_d = nc.declare_dram_parameter("v_static", [4, 128, NCHUNKS, 65], BF, isOutput=False)
    cosk_d = nc.declare_dram_parameter("cos_k", [128, KVP], FP, isOutput=False)
    sink_d = nc.declare_dram_parameter("sin_k", [128, KVP], FP, isOutput=False)
    cosq_d = nc.declare_dram_parameter("cos_q", [128, N], FP, isOutput=False)
    sinq_d = nc.declare_dram_parameter("sin_q", [128, N], FP, isOutput=False)
    masks_d = nc.declare_dram_parameter("masks", [4, 128, 512], BF, isOutput=False)

    outrs_d = nc.declare_dram_parameter("out_rs", [512, DIM], FP, isOutput=True)
    kout_d = nc.declare_dram_parameter("k_out", [2, 128, N], FP, isOutput=True)
    vout_d = nc.declare_dram_parameter("v_out", [128, 16, 256], FP, isOutput=True)

    with tile.TileContext(nc) as tc, \
         tc.tile_pool(name="persist", bufs=1) as persist, \
         tc.tile_pool(name="dram", bufs=1, space="DRAM") as dram, \
         tc.tile_pool(name="xt", bufs=1) as xtp, \
         tc.tile_pool(name="stream", bufs=2) as stream, \
         tc.tile_pool(name="wpool", bufs=1) as wp, \
         tc.tile_pool(name="tab", bufs=2) as tabp, \
         tc.tile_pool(name="krawp", bufs=1) as krawp, \
         tc.tile_pool(name="exps", bufs=6) as expp, \
         tc.tile_pool(name="nrm", bufs=1) as nrmp, \
         tc.tile_pool(name="ost", bufs=3) as ostp, \
         tc.tile_pool(name="ps", bufs=2, space="PSUM") as ps, \
         tc.tile_pool(name="qkps", bufs=4, space="PSUM") as qkps, \
         tc.tile_pool(name="avps", bufs=2, space="PSUM") as avps:

        qT = persist.tile([128, 2, N], BF)
        krot = persist.tile([128, 2, KVP], BF)
        vkv = persist.tile([128, 4, NCHUNKS, 65], BF)
        aoT = persist.tile([128, 2, N], DT_MM)
        masks_sb = persist.tile([128, 4, 512], BF)
        nc.sync.dma_start(vkv[:], vst_d.rearrange("h p c d -> p h c d"))
        nc.sync.dma_start(masks_sb[:], masks_d.rearrange("d p f -> p d f"))

        part_out = dram.tile([N, DIM], FP)
        rs_out = dram.tile([512, DIM], FP)

        ident = wp.tile([128, 128], FP)
        make_identity(nc, ident)
        wq_sb = wp.tile([128, 8, 256], DT_MM)
        wk_sb = wp.tile([128, 8, 256], DT_MM)
        wv_sb = wp.tile([128, 8, 256], DT_MM)
        wout_sb = wp.tile([128, 2, DIM], DT_MM)
        nc.sync.dma_start(wq_sb[:], wq_d.rearrange("(dc p) c -> p dc c", p=128))
        nc.sync.dma_start(wk_sb[:], wk_d.rearrange("(dc p) c -> p dc c", p=128))
        nc.sync.dma_start(wv_sb[:], wv_d.rearrange("(dc p) c -> p dc c", p=128))
        nc.sync.dma_start(wout_sb[:], wout_d.rearrange("(hp p) n -> p hp n", p=128))

        # PE ordering chain: keeps same-stationary matmuls adjacent and lets us
        # splice projection groups into the attention stream as filler work.
        _pe_chain = [None]

        def chain_pe(mm):
            if _pe_chain[0] is not None:
                add_dep_helper(mm.ins, _pe_chain[0].ins, sync=False,
                               reason="pe order")
            _pe_chain[0] = mm

        # ---- x transpose (PE transpose-mode, 128x128 tiles)
        xT = xtp.tile([128, 8, N], DT_MM)
        _tpools = [(ps, "ps"), (qkps, "qk"), (avps, "av")]
        for nt in range(16):
            xa = stream.tile([128, DIM], FP, tag="xa")
            nc.sync.dma_start(xa[:], x_d[128 * nt:128 * nt + 128, :])
            for dc in range(8):
                pool, tg = _tpools[(nt * 8 + dc) % 3]
                pt = pool.tile([128, 512], FP, tag=tg, name=f"tp{nt}_{dc}")
                nc.tensor.transpose(pt[0:128, 0:128], xa[:, 128 * dc:128 * dc + 128], ident[:])
                nc.vector.tensor_copy(xT[:, dc, 128 * nt:128 * nt + 128], pt[0:128, 0:128])

        def rot_half(dst, src):
            # dst = rotate_half(src) along the dh-partition axis, per head
            for a in (0, 1):
                r = 64 * a
                nc.vector.tensor_scalar_mul(dst[r:r + 32, :], src[r + 32:r + 64, :], -1.0)
                nc.vector.tensor_copy(dst[r + 32:r + 64, :], src[r:r + 32, :])

        # ---- v projection (all 4 heads; also the fp32 next_v output)
        for nt in range(16):
            pv = ps.tile([128, 512], FP, tag="ps", name=f"pv{nt}")
            for dc in range(8):
                mm = nc.tensor.matmul(
                    pv[:, :256],
                    xT[:, dc, 128 * nt:128 * nt + 128].bitcast(DT_MM),
                    wv_sb[:, dc, :].bitcast(DT_MM),
                    start=(dc == 0), stop=(dc == 7),
                )
            for a in range(4):
                nc.vector.tensor_copy(vkv[:, a, 4 + nt, 0:DH], pv[:, 64 * a:64 * a + 64])
            vst = stream.tile([128, 256], FP, tag="vst")
            nc.vector.tensor_copy(vst[:], pv[:, :256])
            nc.sync.dma_start(vout_d[:, nt, :], vst[:])

        def q_group(hp, ni, chained):
            ns = slice(512 * ni, 512 * ni + 512)
            pq = ps.tile([128, 512], FP, tag="ps", name=f"pq{hp}_{ni}")
            for dc in range(8):
                mm = nc.tensor.matmul(
                    pq[:],
                    wq_sb[:, dc, 128 * hp:128 * hp + 128].bitcast(DT_MM),
                    xT[:, dc, ns].bitcast(DT_MM),
                    start=(dc == 0), stop=(dc == 7),
                )
                if chained:
                    chain_pe(mm)
            cos_t = tabp.tile([128, 512], FP, tag="cos")
            sin_t = tabp.tile([128, 512], FP, tag="sin")
            nc.sync.dma_start(cos_t[:], cosq_d[:, ns])
            nc.sync.dma_start(sin_t[:], sinq_d[:, ns])
            rot = stream.tile([128, 512], FP, tag="rot")
            rot_half(rot, pq)
            t1 = stream.tile([128, 512], FP, tag="t1")
            nc.vector.tensor_mul(t1[:], pq[:], cos_t[:])
            nc.vector.tensor_mul(rot[:], rot[:], sin_t[:])
            nc.vector.tensor_add(qT[:, hp, ns], t1[:], rot[:])

        def k_group(hp, ni, kraw, chained):
            ks = slice(512 + 512 * ni, 512 + 512 * ni + 512)
            pk = ps.tile([128, 512], FP, tag="ps", name=f"pk{hp}_{ni}")
            for dc in range(8):
                mm = nc.tensor.matmul(
                    pk[:],
                    wk_sb[:, dc, 128 * hp:128 * hp + 128].bitcast(DT_MM),
                    xT[:, dc, slice(512 * ni, 512 * ni + 512)].bitcast(DT_MM),
                    start=(dc == 0), stop=(dc == 7),
                )
                if chained:
                    chain_pe(mm)
            nc.vector.tensor_copy(kraw[:, ks], pk[:])
            cos_t = tabp.tile([128, 512], FP, tag="cos")
            sin_t = tabp.tile([128, 512], FP, tag="sin")
            nc.sync.dma_start(cos_t[:], cosk_d[:, ks])
            nc.sync.dma_start(sin_t[:], sink_d[:, ks])
            rot = stream.tile([128, 512], FP, tag="rot")
            rot_half(rot, pk)
            t1 = stream.tile([128, 512], FP, tag="t1")
            nc.vector.tensor_mul(t1[:], pk[:], cos_t[:])
            nc.vector.tensor_mul(rot[:], rot[:], sin_t[:])
            nc.vector.tensor_add(krot[:, hp, ks], t1[:], rot[:])

        def k_mem_rotary(hp, kraw):
            # rotary for the mem [0:512] and null+pad [2560:2688] regions, and
            # the raw-k output DMA; DVE/DMA only, no PE work.
            for lo, ln in ((0, 512), (2560, 128)):
                rs = slice(lo, lo + ln)
                cos_t = tabp.tile([128, 512], FP, tag="cos")
                sin_t = tabp.tile([128, 512], FP, tag="sin")
                nc.sync.dma_start(cos_t[:, :ln], cosk_d[:, rs])
                nc.sync.dma_start(sin_t[:, :ln], sink_d[:, rs])
                rot = stream.tile([128, 512], FP, tag="rot")
                rot_half(rot[:, :ln], kraw[:, rs])
                t1 = stream.tile([128, 512], FP, tag="t1")
                nc.vector.tensor_mul(t1[:, :ln], kraw[:, rs], cos_t[:, :ln])
                nc.vector.tensor_mul(rot[:, :ln], rot[:, :ln], sin_t[:, :ln])
                nc.vector.tensor_add(krot[:, hp, rs], t1[:, :ln], rot[:, :ln])
            nc.sync.dma_start(kout_d[hp], kraw[:, 512:512 + N])

        def chunk_nis(c, nis):
            if c == 20:
                return list(nis)
            return [ni for ni in nis if c <= min(4 * ni + 7, 19)]

        def emit_qk_batch(hp, hr, c, nis):
            cs = slice(128 * c, 128 * c + 128)
            exs = {}
            for ni in chunk_nis(c, nis):
                ns = slice(512 * ni, 512 * ni + 512)
                qk = qkps.tile([128, 512], FP, tag="qk", name=f"qk{hp}_{c}_{ni}")
                mm = nc.tensor.matmul(
                    qk[:], krot[hr, hp, cs], qT[hr, hp, ns],
                    start=True, stop=True,
                )
                chain_pe(mm)
                ex = expp.tile([128, 512], BF, tag="ex")
                nc.scalar.activation(
                    ex[:], qk[:], mybir.ActivationFunctionType.Exp, scale=SCALE,
                )
                d = c - 4 * ni
                if 4 <= d <= 7 and c <= 19:
                    nc.vector.tensor_mul(ex[:], ex[:], masks_sb[:, d - 4, :])
                exs[ni] = ex[:]
            return exs

        def emit_av_batch(hp, a, c, av, exs):
            for ni in chunk_nis(c, list(av)):
                mm = nc.tensor.matmul(
                    av[ni][0:65, :],
                    vkv[:, 2 * hp + a, c, :],
                    exs[ni],
                    start=(c == 0), stop=(c == 20),
                    skip_group_check=True,
                )
                chain_pe(mm)

        def attention(hp, a, nis, fillers):
            # one ni-pass (two accumulators) of attention for one head
            hr = slice(64 * a, 64 * a + 64)
            av = {ni: avps.tile([128, 512], FP, tag="av", name=f"av{hp}_{a}_{ni}")
                  for ni in nis}
            last_c = min(4 * max(nis) + 7, 19)
            chunks = list(range(last_c + 1)) + [20]
            pending = []
            for c in chunks:
                exs = emit_qk_batch(hp, hr, c, nis)
                pending.append((c, exs))
                # AV lags two chunks behind QK: the exp of a chunk has two full
                # QK batches of time to complete before its AV needs it
                if len(pending) > 2:
                    cc, ee = pending.pop(0)
                    emit_av_batch(hp, a, cc, av, ee)
                if fillers and c % 2 == 1:
                    fillers.pop(0)()
            for cc, ee in pending:
                emit_av_batch(hp, a, cc, av, ee)
            # paired normalization: one broadcast+reciprocal for both ni
            den = nrmp.tile([1, 1024], FP, tag="den")
            for j, ni in enumerate(nis):
                nc.vector.tensor_copy(den[0:1, 512 * j:512 * j + 512], av[ni][64:65, :])
            bc = nrmp.tile([64, 1024], FP, tag="bc")
            nc.gpsimd.partition_broadcast(bc[:], den[0:1, :])
            rec = nrmp.tile([64, 1024], FP, tag="rec")
            nc.vector.reciprocal(rec[:], bc[:])
            for j, ni in enumerate(nis):
                ns = slice(512 * ni, 512 * ni + 512)
                nc.vector.tensor_mul(
                    aoT[64 * a:64 * a + 64, hp, ns], av[ni][0:64, :],
                    rec[:, 512 * j:512 * j + 512]
                )

        # head pair 0 projections (unchained: PE is otherwise idle here)
        kraw0 = krawp.tile([128, KVP], FP, tag="kraw", name="kraw0")
        nc.sync.dma_start(kraw0[:], kst_d[0])
        for ni in range(NI):
            q_group(0, ni, chained=False)
        for ni in range(NI):
            k_group(0, ni, kraw0, chained=False)
        k_mem_rotary(0, kraw0)

        # head pair 1 projections are spliced into attention(0, *) as fillers
        kraw1 = krawp.tile([128, KVP], FP, tag="kraw", name="kraw1")
        nc.sync.dma_start(kraw1[:], kst_d[1])
        fillers = [lambda ni=ni: q_group(1, ni, chained=True) for ni in range(NI)]
        fillers += [lambda ni=ni: k_group(1, ni, kraw1, chained=True) for ni in range(NI)]

        def out_proj_half(half):
            # output projection for rows [1024*half, 1024*half + 1024) plus the
            # ReduceScatter of that half; the first half's RS overlaps pass B.
            for nt in range(8 * half, 8 * half + 8):
                for ncol in range(2):
                    po = ps.tile([128, 512], FP, tag="ps", name=f"po{nt}_{ncol}")
                    for hp in range(2):
                        nc.tensor.matmul(
                            po[:],
                            aoT[:, hp, 128 * nt:128 * nt + 128].bitcast(DT_MM),
                            wout_sb[:, hp, 512 * ncol:512 * ncol + 512].bitcast(DT_MM),
                            start=(hp == 0), stop=(hp == 1),
                        )
                    ot = ostp.tile([128, 512], FP, tag="ot")
                    nc.vector.tensor_copy(ot[:], po[:])
                    nc.sync.dma_start(
                        part_out[128 * nt:128 * nt + 128, 512 * ncol:512 * ncol + 512],
                        ot[:],
                    )
            nc.gpsimd.collective_compute(
                "ReduceScatter",
                mybir.AluOpType.add,
                replica_groups=[[0, 1, 2, 3], [4, 5, 6, 7]],
                ins=[part_out[1024 * half:1024 * half + 1024, :]],
                outs=[rs_out[256 * half:256 * half + 256, :]],
            )
            nc.sync.dma_start(outrs_d[256 * half:256 * half + 256, :],
                              rs_out[256 * half:256 * half + 256, :])

        # pass A: ni 0,1 for all heads (fillers: head-pair-1 projections)
        attention(0, 0, (0, 1), fillers)
        attention(0, 1, (0, 1), fillers)
        k_mem_rotary(1, kraw1)
        attention(1, 0, (0, 1), fillers)
        attention(1, 1, (0, 1), fillers)
        out_proj_half(0)
        # pass B: ni 2,3 (RS of half 0 overlaps this pass)
        attention(0, 0, (2, 3), fillers)
        attention(0, 1, (2, 3), fillers)
        attention(1, 0, (2, 3), fillers)
        attention(1, 1, (2, 3), fillers)
        out_proj_half(1)

    nc.compile()
    return nc


def _host_inputs(x, rotary_q, rotary_k, xl_memories, Wq, Wkv, Wout, null_kv):
    """Build the 8 per-core input dicts."""
    x = np.ascontiguousarray(x, np.float32)
    cos_q_T = np.cos(rotary_q).T.astype(np.float32)   # [64, 2048]
    sin_q_T = np.sin(rotary_q).T.astype(np.float32)
    # permuted kv order: mem(512) | seq(2048) | null(1) | pad(127)
    angles = np.concatenate(
        [rotary_k[0:M], rotary_k[M + 1:KV], rotary_k[M:M + 1],
         np.zeros((KVP - KV, DH), np.float32)], axis=0)
    cos_k_T = np.cos(angles).T.astype(np.float32)     # [64, 2688]
    sin_k_T = np.sin(angles).T.astype(np.float32)
    cos_k_T[:, KV:] = 0.0
    sin_k_T[:, KV:] = 0.0
    cos_q2 = np.concatenate([cos_q_T, cos_q_T], axis=0)  # [128, 2048]
    sin_q2 = np.concatenate([sin_q_T, sin_q_T], axis=0)
    cos_k2 = np.concatenate([cos_k_T, cos_k_T], axis=0)  # [128, 2688]
    sin_k2 = np.concatenate([sin_k_T, sin_k_T], axis=0)

    p = np.arange(128)[:, None]
    f = np.arange(512)[None, :]
    masks = np.stack(
        [(512 - 128 * d - p + f >= 0).astype(np.float32) for d in range(4, 8)])

    ins = []
    for c in range(8):
        b, hg = c // 4, c % 4
        H0 = 4 * hg
        k_static = np.zeros((2, 128, KVP), np.float32)
        v_static = np.zeros((4, 128, NCHUNKS, 65), np.float32)
        for hp in range(2):
            for a in range(2):
                h = H0 + 2 * hp + a
                r = slice(64 * a, 64 * a + 64)
                k_static[hp, r, 0:M] = xl_memories[0][b, h].T
                k_static[hp, r, M + N] = null_kv[0][h]
        for a in range(4):
            h = H0 + a
            v_static[a, :, :, 64] = 1.0
            v_static[a, :, 20, :] = 0.0
            v_static[a, :, 0:4, 0:DH] = (
                xl_memories[1][b, h].reshape(4, 128, DH).transpose(1, 0, 2))
            v_static[a, 0, 20, 0:DH] = null_kv[1][h]
            v_static[a, 0, 20, 64] = 1.0
        cs = slice(64 * H0, 64 * H0 + 256)
        ins.append({
            "x": x[b],
            "wq": np.ascontiguousarray(Wq[:, cs], np.float32),
            "wk": np.ascontiguousarray(Wkv[:, 0:INNER][:, cs], np.float32),
            "wv": np.ascontiguousarray(Wkv[:, INNER:][:, cs], np.float32),
            "wout": np.ascontiguousarray(Wout[cs, :], np.float32),
            "k_static": k_static,
            "v_static": v_static.astype(ml_dtypes.bfloat16),
            "cos_k": cos_k2, "sin_k": sin_k2,
            "cos_q": cos_q2, "sin_q": sin_q2,
            "masks": masks.astype(ml_dtypes.bfloat16),
        })
    return ins


def kernel(x, rotary_q, rotary_k, xl_memories, Wq, Wkv, Wout, null_kv):
    global _COMPILED
    x = np.asarray(x, np.float32)
    rotary_q = np.asarray(rotary_q, np.float32)
    rotary_k = np.asarray(rotary_k, np.float32)
    xl_memories = np.asarray(xl_memories, np.float32)
    Wq = np.asarray(Wq, np.float32)
    Wkv = np.asarray(Wkv, np.float32)
    Wout = np.asarray(Wout, np.float32)
    null_kv = np.asarray(null_kv, np.float32)

    if _COMPILED is None:
        _COMPILED = _build_kernel()
    nc = _COMPILED

    ins = _host_inputs(x, rotary_q, rotary_k, xl_memories, Wq, Wkv, Wout, null_kv)
    global _LAST_RESULT
    _LAST_RESULT = run_bass_kernel_spmd(nc, ins, list(range(8)))
    res = _LAST_RESULT.results

    out = np.empty((B, N, INNER), np.float32)
    next_k = np.empty((B, H, 1 + N, DH), np.float32)
    next_v = np.empty((B, H, 1 + N, DH), np.float32)
    next_k[:, :, 0, :] = null_kv[0][None]
    next_v[:, :, 0, :] = null_kv[1][None]
    for c in range(8):
        b, hg = c // 4, c % 4
        H0 = 4 * hg
        orow = res[c]["out_rs"]
        out[b, 256 * hg:256 * hg + 256, :] = orow[0:256]
        out[b, 1024 + 256 * hg:1024 + 256 * hg + 256, :] = orow[256:512]
        ko = res[c]["k_out"].reshape(2, 2, DH, N)     # [hp, a, d, n]
        for hp in range(2):
            for a in range(2):
                next_k[b, H0 + 2 * hp + a, 1:, :] = ko[hp, a].T
        vo = res[c]["v_out"]                          # [128, 16, 4*64]
        for a in range(4):
            next_v[b, H0 + a, 1:, :] = (
                vo[:, :, 64 * a:64 * a + 64].transpose(1, 0, 2).reshape(N, DH))
    return out, np.stack([next_k, next_v])
